# revision 5
# baseline (speedup 1.0000x reference)
"""GQA kernel v4 for TRN2, 8 NeuronCores.

v4 vs v3: every matmul stationary operand is a separate offset-0 SBUF
tile (column-sliced stationary operands measured 10-20x slower per MM).

Sharding: DP2 (batch) x TP4 (head groups). Core r handles batch b=r//4,
quad q=r%4 -> global Q heads 8q..8q+7 (KV groups 2q, 2q+1).

v3 vs v2:
  - PV matmuls col-packed in pairs via tile_position (M=64+64 -> full
    array concurrency): 2 PSUM banks instead of 4, ~2x PV throughput.
  - Softmax sums no longer ride as a 65th V' row: est tiles are
    accumulated on DVE into esum, then one M=1 matmul per slot computes
    the column sums into a shared PSUM bank (partitions 0/32/64/96).
  - Normalize drains po banks to SBUF quickly (frees them for the next
    half), then broadcasts 1/sum back into the po banks via K=1 matmuls.
  - P4 (output projection) inlined after each chunk's AllGather with a
    single PSUM bank -> overlaps the next chunk's attention.

PSUM budget: s0,s1 scores (2+2) + poA,poB (1+1) + sums (1) + py (1) = 8.
"""

import numpy as np

import concourse.bacc as bacc
import concourse.bass as bass
import concourse.mybir as mybir
import concourse.tile as tile
from concourse.bass_utils import run_bass_kernel_spmd
from concourse.masks import make_identity

D = 2048          # d_model
H = 32            # query heads
G = 8             # kv groups
DK = 64           # head dim
B = 2
S = 2048          # tokens per batch
TOK = S
NCORES = 8
NQ = 4            # quads (TP degree)
LH = 8            # local query heads per core
QDIM = LH * DK    # 512 local q dims
KVDIM = 2 * DK    # 128 local kv dims (2 groups)
WCOLS = QDIM + KVDIM + KVDIM  # 768 packed projection cols
NKT = D // 128    # 16 contraction tiles over d_model
NTT = TOK // 128  # 16 token tiles of 128
NC512 = TOK // 512  # 4 token chunks of 512

F32 = mybir.dt.float32
F16 = mybir.dt.float16
EXP = mybir.ActivationFunctionType.Exp


def _build_nc(repeat: int = 1) -> bass.Bass:
    nc = bacc.Bacc("TRN2", num_devices=NCORES)

    xt_d = nc.dram_tensor("xt", [D, TOK], F16, kind="ExternalInput")
    wqkv_d = nc.dram_tensor("wqkv", [D, WCOLS], F16, kind="ExternalInput")
    bqkv_d = nc.dram_tensor("bqkv", [WCOLS], F32, kind="ExternalInput")
    wo_d = nc.dram_tensor("wo", [D, QDIM], F16, kind="ExternalInput")
    bo_d = nc.dram_tensor("bo", [QDIM], F32, kind="ExternalInput")
    yt_d = nc.dram_tensor("yt", [QDIM, TOK], F32, kind="ExternalOutput")

    # context staging: one dram tensor per 512-token chunk so the
    # chunked AllGathers carry no false cross-chunk dependencies
    ct_src = [nc.dram_tensor(f"ct_src{c}", [QDIM, 512], F16)
              for c in range(NC512)]
    ct_all = [nc.dram_tensor(f"ct_all{c}", [D, 512], F16)
              for c in range(NC512)]
    replica_groups = [[0, 1, 2, 3], [4, 5, 6, 7]]

    from contextlib import ExitStack
    with tile.TileContext(nc) as tc:
        with ExitStack() as _stk:
            persist = _stk.enter_context(tc.tile_pool(name="persist", bufs=1))
            qtp = _stk.enter_context(tc.tile_pool(name="qt", bufs=1))
            ktvp = _stk.enter_context(tc.tile_pool(name="ktv", bufs=1))
            wpool = _stk.enter_context(tc.tile_pool(name="wq", bufs=1))
            wop = _stk.enter_context(tc.tile_pool(name="wo", bufs=1))
            vtpool = _stk.enter_context(tc.tile_pool(name="vt", bufs=1))
            xpool = _stk.enter_context(tc.tile_pool(name="xin", bufs=4))
            stp = _stk.enter_context(tc.tile_pool(name="st", bufs=2))
            esump = _stk.enter_context(tc.tile_pool(name="esum", bufs=2))
            nrmp = _stk.enter_context(tc.tile_pool(name="nrm", bufs=2))
            ctxp = _stk.enter_context(tc.tile_pool(name="ctx", bufs=2))
            cinp = _stk.enter_context(tc.tile_pool(name="cin", bufs=4))
            youtp = _stk.enter_context(tc.tile_pool(name="yout", bufs=4))

            # persistent SBUF tensors
            qt = [qtp.tile([128, TOK], F16, tag=f"qt{t}", name=f"qt{t}")
                  for t in range(4)]
            ktk = [ktvp.tile([128, 128], F16, tag=f"kt{k}", name=f"kt{k}")
                   for k in range(NTT)]
            # V' per (group, k-tile): [128 tok, 64] fp16 (no ones column)
            vpk = [[ktvp.tile([128, DK], F16, tag=f"vp{j}_{k}",
                              name=f"vp{j}_{k}") for k in range(NTT)]
                   for j in range(2)]
            ones_col = ktvp.tile([128, 1], F16, tag="onec", name="onec")
            bias6 = persist.tile([128, 6], F32)
            bo_t = persist.tile([128, 4], F32)
            ones16 = persist.tile([128, 64], F16)
            ident = persist.tile([128, 64], F16)

            nc.vector.memset(ones16[:], 1.0)
            nc.vector.memset(ones_col[:], 1.0)
            make_identity(nc, ident[0:64, :])
            make_identity(nc, ident[64:128, :])
            for m in range(6):
                nc.sync.dma_start(bias6[:, m : m + 1],
                                  bqkv_d[bass.ts(m, 128)].unsqueeze(1))
            for m in range(4):
                nc.sync.dma_start(bo_t[:, m : m + 1],
                                  bo_d[bass.ts(m, 128)].unsqueeze(1))

            for _rep in range(repeat):
                # ---------------- P1: projections ----------------
                with ExitStack() as _s1:
                    pproj = _s1.enter_context(
                        tc.tile_pool(name="pproj", bufs=1, space="PSUM"))
                    ptr = _s1.enter_context(
                        tc.tile_pool(name="ptr", bufs=2, space="PSUM"))

                    w_tiles = []
                    for k in range(NKT):
                        w_tiles.append([
                            wpool.tile([128, 128], F16, tag=f"w{k}_{m}",
                                       name=f"w{k}_{m}") for m in range(6)])

                    vt_sb = vtpool.tile([128, TOK], F16, tag="vt", name="vt")

                    for nc5 in range(NC512):
                        ps = [pproj.tile([128, 512], F32, tag=f"p{m}",
                                         name=f"p{m}") for m in range(6)]
                        for k in range(NKT):
                            if nc5 == 0:
                                for m in range(6):
                                    nc.sync.dma_start(
                                        w_tiles[k][m][:],
                                        wqkv_d[bass.ts(k, 128), bass.ts(m, 128)])
                            xt_t = xpool.tile([128, 512], F16, tag="x", name="x")
                            nc.sync.dma_start(
                                xt_t[:], xt_d[bass.ts(k, 128), bass.ts(nc5, 512)])
                            for m in range(6):
                                nc.tensor.matmul(
                                    ps[m][:], w_tiles[k][m][:], xt_t[:],
                                    start=(k == 0), stop=(k == NKT - 1))
                        for m in range(4):  # Q -> fp16
                            nc.vector.tensor_scalar_add(
                                qt[m][:, bass.ts(nc5, 512)], ps[m][:],
                                bias6[:, m : m + 1])
                        for kk in range(4):  # K -> fp16 per-k tiles
                            nc.vector.tensor_scalar_add(
                                ktk[4 * nc5 + kk][:],
                                ps[4][:, bass.ts(kk, 128)], bias6[:, 4:5])
                        nc.vector.tensor_scalar_add(  # V -> fp16 staging
                            vt_sb[:, bass.ts(nc5, 512)], ps[5][:], bias6[:, 5:6])

                    # V' build: PE transpose [64,128] -> [128,64] fp16
                    for j in range(2):
                        for tt in range(NTT):
                            ps_t = ptr.tile([128, 64], F16, tag="tr", name="tr")
                            nc.tensor.transpose(
                                ps_t[:],
                                vt_sb[bass.ts(j, 64), bass.ts(tt, 128)],
                                ident[bass.ts(j, 64), :])
                            nc.vector.tensor_copy(vpk[j][tt][:], ps_t[:])

                # ---------------- P2 + P4, chunk-pipelined ----------------
                with ExitStack() as _s2:
                    psc = _s2.enter_context(
                        tc.tile_pool(name="psc", bufs=1, space="PSUM"))
                    pov = _s2.enter_context(
                        tc.tile_pool(name="pov", bufs=1, space="PSUM"))
                    psm = _s2.enter_context(
                        tc.tile_pool(name="psm", bufs=1, space="PSUM"))
                    pyp = _s2.enter_context(
                        tc.tile_pool(name="py", bufs=1, space="PSUM"))

                    wo_tiles = []
                    for k in range(NKT):
                        row = []
                        for m in range(4):
                            wt = wop.tile([128, 128], F16, tag=f"wo{k}_{m}",
                                          name=f"wo{k}_{m}")
                            nc.sync.dma_start(
                                wt[:], wo_d[bass.ts(k, 128), bass.ts(m, 128)])
                            row.append(wt)
                        wo_tiles.append(row)

                    def emit_p4(c4):
                        # output projection for chunk c4 (single PSUM bank)
                        ct_in = []
                        for k in range(NKT):
                            ct_t = cinp.tile([128, 512], F16, tag=f"ci{k}",
                                             name=f"ci{k}")
                            nc.sync.dma_start(
                                ct_t[:], ct_all[c4][bass.ts(k, 128), :])
                            ct_in.append(ct_t)
                        for m in range(4):
                            psy = pyp.tile([128, 512], F32, tag="py", name="py")
                            for k in range(NKT):
                                nc.tensor.matmul(
                                    psy[:], wo_tiles[k][m][:], ct_in[k][:],
                                    start=(k == 0), stop=(k == NKT - 1))
                            yo = youtp.tile([128, 512], F32, tag="yo",
                                            name="yo")
                            nc.vector.tensor_scalar_add(
                                yo[:], psy[:], bo_t[:, m : m + 1])
                            nc.sync.dma_start(
                                yt_d[bass.ts(m, 128), bass.ts(c4, 512)], yo[:])

                    for nc5 in range(NC512):
                        for half in range(2):
                            # slots s=0..3: (bank, pos) x (i, j):
                            # s0=(A,top)=(i0,j0) s1=(A,bot)=(i1,j1)
                            # s2=(B,top)=(i1,j0) s3=(B,bot)=(i0,j1)
                            poA = pov.tile([128, 512], F32, tag="poA", name="poA")
                            poB = pov.tile([128, 512], F32, tag="poB", name="poB")
                            esm = [esump.tile([128, 1024], F16, tag=f"es{i}",
                                              name=f"es{i}") for i in range(2)]
                            for k in range(NTT):
                                sts = []
                                for i in range(2):
                                    t = 2 * half + i
                                    s2 = psc.tile([128, 1024], F32, tag=f"s{i}",
                                                  name=f"s{i}")
                                    nc.tensor.matmul(
                                        s2[:, 0:512],
                                        ktk[k][0:64, :],
                                        qt[t][0:64, bass.ts(nc5, 512)],
                                        start=True, stop=True,
                                        tile_position=(0, 0))
                                    nc.tensor.matmul(
                                        s2[:, 512:1024],
                                        ktk[k][64:128, :],
                                        qt[t][64:128, bass.ts(nc5, 512)],
                                        start=True, stop=True,
                                        tile_position=(64, 0))
                                    sts.append(s2)
                                est = []
                                for i in range(2):
                                    e2 = stp.tile([128, 1024], F16, tag=f"e{i}",
                                                  name=f"e{i}")
                                    nc.scalar.activation(e2[:], sts[i][:], EXP)
                                    est.append(e2)
                                    if k == 0:
                                        nc.vector.tensor_copy(esm[i][:], e2[:])
                                    else:
                                        nc.vector.tensor_add(
                                            esm[i][:], esm[i][:], e2[:])
                                # PV pairs: (top: vp0/colgrp0, bot: vp1/colgrp2)
                                vk0 = vpk[0][k][:]
                                vk1 = vpk[1][k][:]
                                nc.tensor.matmul(
                                    poA[0:64, :], vk0, est[0][:, 0:512],
                                    start=(k == 0), stop=(k == NTT - 1),
                                    tile_position=(0, 0))
                                nc.tensor.matmul(
                                    poA[64:128, :], vk1, est[1][:, 512:1024],
                                    start=(k == 0), stop=(k == NTT - 1),
                                    tile_position=(0, 64))
                                nc.tensor.matmul(
                                    poB[0:64, :], vk0, est[1][:, 0:512],
                                    start=(k == 0), stop=(k == NTT - 1),
                                    tile_position=(0, 0))
                                nc.tensor.matmul(
                                    poB[64:128, :], vk1, est[0][:, 512:1024],
                                    start=(k == 0), stop=(k == NTT - 1),
                                    tile_position=(0, 64))

                            # sums: M=1 matmuls into sm partitions 0/32/64/96
                            sm = psm.tile([128, 512], F32, tag="sm", name="sm")
                            slot_ij = [(0, 0), (1, 1), (1, 0), (0, 1)]
                            for s, (i, j) in enumerate(slot_ij):
                                nc.tensor.matmul(
                                    sm[32 * s : 32 * s + 1, :],
                                    ones_col[:],
                                    esm[i][:, bass.ts(j, 512)],
                                    start=True, stop=True,
                                    tile_position=(0, 32 * s))
                            rcpT = nrmp.tile([128, 512], F16, tag="rcp",
                                             name="rcp")
                            for s in range(4):
                                with nc.allow_low_precision(
                                        reason="softmax denom f16"):
                                    nc.vector.reciprocal(
                                        rcpT[32 * s : 32 * s + 1, :],
                                        sm[32 * s : 32 * s + 1, :])
                            # drain po banks -> SBUF (frees them fast)
                            cxA = nrmp.tile([128, 512], F32, tag="cxA",
                                            name="cxA")
                            cxB = nrmp.tile([128, 512], F32, tag="cxB",
                                            name="cxB")
                            nc.vector.tensor_copy(cxA[:], poA[:])
                            nc.vector.tensor_copy(cxB[:], poB[:])
                            # broadcast 1/sum back into po banks (K=1 matmuls)
                            for s, (bank, lo) in enumerate(
                                    [(poA, 0), (poA, 64), (poB, 0), (poB, 64)]):
                                nc.tensor.matmul(
                                    bank[lo : lo + 64, :],
                                    ones16[32 * s : 32 * s + 1, :],
                                    rcpT[32 * s : 32 * s + 1, :],
                                    start=True, stop=True,
                                    tile_position=(32 * s, lo))
                            # normalize + stage context
                            for bank, cx, sl, sh in ((poA, cxA, 0, 1),
                                                     (poB, cxB, 2, 3)):
                                ct_t = ctxp.tile([128, 512], F16, tag="ct",
                                                 name="ct")
                                nc.vector.tensor_mul(ct_t[:], cx[:], bank[:])
                                for s, lo in ((sl, 0), (sh, 64)):
                                    i, j = slot_ij[s]
                                    lhead = 2 * half + i + 4 * j
                                    nc.sync.dma_start(
                                        ct_src[nc5][bass.ts(lhead, 64), :],
                                        ct_t[lo : lo + 64, :])
                        # chunked AllGather for this token block
                        if nc5 >= 1:
                            emit_p4(nc5 - 1)
                        nc.gpsimd.collective_compute(
                            "AllGather", mybir.AluOpType.bypass,
                            replica_groups=replica_groups,
                            ins=[ct_src[nc5][:]],
                            outs=[ct_all[nc5][:]])
                    emit_p4(NC512 - 1)


    nc.compile()
    return nc


_NC_CACHE = {}


def _get_nc(repeat: int = 1):
    if repeat not in _NC_CACHE:
        _NC_CACHE[repeat] = _build_nc(repeat)
    return _NC_CACHE[repeat]


def _prep_core_inputs(x, Wq, bq, Wk, bk, Wv, bv, Wo, bo, core):
    b, q = divmod(core, NQ)
    xt = np.ascontiguousarray(x[b].T).astype(np.float16)  # [D, TOK]

    # local head order: pairs (t, t+4) interleaved -> [0,4,1,5,2,6,3,7]
    # local head L (0..7) = global head 8q+L; groups: L0-3 -> g0, L4-7 -> g1
    head_order = [0, 4, 1, 5, 2, 6, 3, 7]
    qcols = []
    for L in head_order:
        gh = 8 * q + L
        qcols.extend(range(gh * DK, (gh + 1) * DK))
    kv0 = 2 * q * DK  # global kv dim offset for g0
    kvcols = list(range(kv0, kv0 + 2 * DK))

    wqkv = np.empty((D, WCOLS), dtype=np.float16)
    wqkv[:, :QDIM] = (Wq[:, qcols] / 8.0).astype(np.float16)
    wqkv[:, QDIM : QDIM + KVDIM] = Wk[:, kvcols].astype(np.float16)
    wqkv[:, QDIM + KVDIM :] = Wv[:, kvcols].astype(np.float16)
    bqkv = np.concatenate([bq[qcols] / 8.0, bk[kvcols], bv[kvcols]]
                          ).astype(np.float32)

    out_lo = 512 * q
    wo = np.ascontiguousarray(Wo[:, out_lo : out_lo + QDIM]).astype(np.float16)
    bo_s = np.ascontiguousarray(bo[out_lo : out_lo + QDIM]).astype(np.float32)

    return {"xt": xt, "wqkv": wqkv, "bqkv": bqkv, "wo": wo, "bo": bo_s}


def kernel(x, Wq, bq, Wk, bk, Wv, bv, Wo, bo, _trace=False):
    args = [np.asarray(a, dtype=np.float32)
            for a in (x, Wq, bq, Wk, bk, Wv, bv, Wo, bo)]
    nc = _get_nc()
    in_maps = [_prep_core_inputs(*args, core) for core in range(NCORES)]
    res = run_bass_kernel_spmd(nc, in_maps, core_ids=list(range(NCORES)),
                               trace=_trace)

    y = np.empty((B, S, D), dtype=np.float32)
    for core in range(NCORES):
        b, q = divmod(core, NQ)
        y[b, :, 512 * q : 512 * (q + 1)] = res.results[core]["yt"].T
    if _trace:
        return y, res
    return y
